# revision 62
# baseline (speedup 1.0000x reference)
"""Trainium2 Bass kernel for nn_CSAModule_47768626266174.

Mathematical structure of the reference:

    S    = softmax(attn, axis=-1)                # [C, T, T]
    out  = base + sigma * einsum('bft,ct->bcf', inputs, S.mean(axis=-1))
    base = inputs.mean(-1)[:, None, :]

``S.mean(axis=-1)`` averages over the *same* axis the softmax normalizes,
so it is exactly 1/T for every (c, t) — independent of the attention
contents, the conv weights, and the labels.  Hence

    out[b, c, f] = (1 + sigma) * mean_t inputs[b, f, t]

for every class c.  This identity holds for all finite inputs (softmax is
shift-normalized, rows sum to 1), so the kernel only needs to read
``inputs`` once, reduce over T, scale by (1 + sigma), and broadcast over
the class dim.  That is the true memory roofline of this module.

Sharding: data-parallel over batch B — each of the 8 cores reduces its
8-item chunk; no collectives.  Output chunks are concatenated on host.

Raw Bass (not Tile): this container's walrus build encodes at most ONE
semaphore wait per instruction, which rejects Tile's kernel-tail drain.
Standalone wait_ge instructions carry one condition each; anything
needing several predecessors gets several wait_ge's in front.

Per-core dataflow.  The critical path is the serialized DMA data stream
(~2.1 MB at ~360 GB/s); every other stage is pipelined per batch item
behind it, so the post-stream tail is just the last item's short chain:

  sync  : per-item input DMAs (per-DMA semaphores — dynamic HW queues
          complete out of order; the last item is loaded in two halves so
          the tail reduce is half-size), one store of y as [C, B, F]
  gpsimd: sigma DMA on SWDGE (keeps the HWDGE path free for x),
          ident_s = diag((1+sigma)/T) in one affine_select — all early
  vector: s1 = (1+sigma)/T, T-reduces (594 ns/item < 728 ns arrival),
          the last item's PSUM->SBUF copy
  tensor: K=1 matmul broadcasting s1 across partitions into psc; then per
          item b a small matmul into its own PSUM bank:
          pt_b = (sums[:, b] bcast over C).T @ ident_s  ([C, F] = y rows)
          (the tail item as two PSUM-accumulating half matmuls)
  scalar: per item, PSUM -> SBUF copy of the finished y rows into
          yt [C, B*F] (compute engines can only address partition bases
          0/32/64/96, so items advance along the free axis)
"""

from contextlib import ExitStack

import numpy as np

B, F, T, C = 64, 128, 512, 10
N_CORES = 8
BPC = B // N_CORES  # batch items per core

_NC_CACHE = None


def _build_bass():
    """Build the per-core Bass module (SPMD: same program on all cores)."""
    global _NC_CACHE
    if _NC_CACHE is not None:
        return _NC_CACHE

    import concourse.bass as bass
    import concourse.mybir as mybir

    fp32 = mybir.dt.float32
    nc = bass.Bass()

    x = nc.dram_tensor("x", [BPC, F, T], fp32, kind="ExternalInput")
    sig = nc.dram_tensor("sig", [1, 1], fp32, kind="ExternalInput")
    y = nc.dram_tensor("y", [BPC, C, F], fp32, kind="ExternalOutput")

    with ExitStack() as ctx:
        e = ctx.enter_context
        xt = e(nc.sbuf_tensor("xt", [128, BPC * T], fp32))
        # One extra column: the last item is reduced in two halves (its
        # load is split so the tail reduce is half-size and starts earlier;
        # the PE recombines via PSUM accumulation).
        sums = e(nc.sbuf_tensor("sums", [128, BPC + 1], fp32))
        ident_s = e(nc.sbuf_tensor("ident_s", [128, 128], fp32))
        sg = e(nc.sbuf_tensor("sg", [1, 1], fp32))
        s1 = e(nc.sbuf_tensor("s1", [1, 1], fp32))
        ones_row = e(nc.sbuf_tensor("ones_row", [1, 128], fp32))
        scale_col = e(nc.sbuf_tensor("scale_col", [128, 1], fp32))
        # [C partitions, BPC*F free]: per-item copies land at free-dim
        # offsets (compute engines may only start at partition 0/32/64/96).
        yt = e(nc.sbuf_tensor("yt", [C, BPC * F], fp32))
        # psc is allocated and immediately freed: its bank is reused by
        # pts[0].  Safe because the first per-item matmul waits for the
        # scale_col copy (dve >= 4 > 3), after which psc is dead.
        psc_cm = nc.psum_tensor("psc", [128, 1], fp32)
        psc = psc_cm.__enter__()
        psc_cm.__exit__(None, None, None)
        # One PSUM bank per item: matmul outputs must start at partition
        # 0/32/64, and bank separation means the PE write of item b+1
        # never touches the bank ACT is reading for item b.
        pts = [e(nc.psum_tensor(f"pt{b}", [C, 128], fp32)) for b in range(BPC)]

        # One semaphore per DMA: dynamic HW queues complete out of order.
        sig_sem = e(nc.semaphore("sig_sem"))
        x_sems = [e(nc.semaphore(f"xld{b}")) for b in range(BPC)]
        x_sem_tail = e(nc.semaphore("xld_tail"))
        store_sem = e(nc.semaphore("store_sem"))
        dve_sem = e(nc.semaphore("dve_sem"))
        act_sem = e(nc.semaphore("act_sem"))
        pe_sem = e(nc.semaphore("pe_sem"))
        pool_sem = e(nc.semaphore("pool_sem"))

        block = e(nc.Block())

        # dve_sem milestones: 1 ones_row, 2 s1, 3 scale_col copy,
        # 4+b = reduce of item b done.

        H = T // 2  # tail-item half size

        @block.sync
        def _(sync):
            for b in range(BPC - 1):
                sync.dma_start(
                    xt[:, b * T : (b + 1) * T], x[b, :, :]
                ).then_inc(x_sems[b], 16)
            bl = BPC - 1
            sync.dma_start(
                xt[:, bl * T : bl * T + H], x[bl, :, 0:H]
            ).then_inc(x_sems[bl], 16)
            sync.dma_start(
                xt[:, bl * T + H : (bl + 1) * T], x[bl, :, H:T]
            ).then_inc(x_sem_tail, 16)
            sync.wait_ge(act_sem, BPC - 1)  # yt columns 0..BPC-2 copied
            sync.wait_ge(dve_sem, BPC + 5)  # last yt column copied (DVE)
            sync.dma_start(
                y[:, :, :].rearrange("b c f -> c b f"),
                yt[:, :].rearrange("c (b f) -> c b f", f=F),
            ).then_inc(store_sem, 16)
            sync.wait_ge(store_sem, 16)

        @block.gpsimd
        def _(gpsimd):
            # SWDGE load of sigma — the HWDGE descriptor path stays free
            # for the x stream.
            gpsimd.dma_start(sg[:, :], sig[:, :]).then_inc(sig_sem, 16)
            # ident_s = diag((1+sigma)/T) in a single op: select between a
            # step-0 broadcast of scale_col and 0.0.
            gpsimd.wait_ge(dve_sem, 3)  # scale_col ready
            gpsimd.affine_select(
                out=ident_s[:, :],
                in_=scale_col[:, :].broadcast_to((128, 128)),
                compare_op=mybir.AluOpType.is_equal,
                fill=0.0,
                base=0,
                pattern=[[-1, 128]],
                channel_multiplier=1,
            ).then_inc(pool_sem, 1)  # p1

        @block.vector
        def _(vector):
            vector.memset(ones_row[:, :], 1.0).then_inc(dve_sem, 1)
            vector.wait_ge(sig_sem, 16)
            # s1 = sigma/T + 1/T = (1+sigma)/T
            vector.tensor_scalar(
                out=s1[:, :],
                in0=sg[:, :],
                scalar1=1.0 / T,
                scalar2=1.0 / T,
                op0=mybir.AluOpType.mult,
                op1=mybir.AluOpType.add,
            ).then_inc(dve_sem, 1)
            vector.wait_ge(pe_sem, 1)  # psc ready
            vector.tensor_copy(scale_col[:, :], psc[:, :]).then_inc(dve_sem, 1)
            for b in range(BPC - 1):
                vector.wait_ge(x_sems[b], 16)
                vector.reduce_sum(
                    out=sums[:, b : b + 1],
                    in_=xt[:, b * T : (b + 1) * T],
                    axis=mybir.AxisListType.X,
                ).then_inc(dve_sem, 1)
            # Tail item in two half reduces (partials in cols BPC-1, BPC).
            bl = BPC - 1
            vector.wait_ge(x_sems[bl], 16)
            vector.reduce_sum(
                out=sums[:, bl : bl + 1],
                in_=xt[:, bl * T : bl * T + H],
                axis=mybir.AxisListType.X,
            ).then_inc(dve_sem, 1)  # dve = BPC + 3
            vector.wait_ge(x_sem_tail, 16)
            vector.reduce_sum(
                out=sums[:, bl + 1 : bl + 2],
                in_=xt[:, bl * T + H : (bl + 1) * T],
                axis=mybir.AxisListType.X,
            ).then_inc(dve_sem, 1)  # dve = BPC + 4
            # Last item's PSUM -> SBUF copy on DVE (free after its final
            # reduce, and its copy is ~2x faster than ACT's).
            vector.wait_ge(pe_sem, BPC + 2)
            vector.tensor_copy(
                yt[:, (BPC - 1) * F : BPC * F], pts[BPC - 1][:, :]
            ).then_inc(dve_sem, 1)  # dve = BPC + 5

        @block.tensor
        def _(tensor):
            tensor.wait_ge(dve_sem, 2)  # ones_row + s1
            # psc[p, 0] = (1+sigma)/T on every partition (K=1 matmul).
            tensor.matmul(
                psc[:, :], ones_row[:, :], s1[:, :], start=True, stop=True
            ).then_inc(pe_sem, 1)
            tensor.wait_ge(pool_sem, 1)  # ident_s ready
            # One small matmul per item, issued as its reduce lands:
            # pt_b[c, f] = sums[f, b] * (1+sigma)/T.  lhsT is the item's
            # sums column broadcast over classes via one step-0 free dim;
            # the sigma scale rides the diagonal matrix.
            for b in range(BPC - 1):
                tensor.wait_ge(dve_sem, b + 4)
                tensor.matmul(
                    pts[b][:, :],
                    sums[:, b : b + 1].broadcast_to((128, C)),
                    ident_s[:, :],
                    start=True,
                    stop=True,
                ).then_inc(pe_sem, 1)  # pe = b + 2
            # Tail item: two accumulating matmuls over its half-reduces.
            bl = BPC - 1
            tensor.wait_ge(dve_sem, BPC + 3)
            tensor.matmul(
                pts[bl][:, :],
                sums[:, bl : bl + 1].broadcast_to((128, C)),
                ident_s[:, :],
                start=True,
                stop=False,
            ).then_inc(pe_sem, 1)  # pe = BPC + 1
            tensor.wait_ge(dve_sem, BPC + 4)
            tensor.matmul(
                pts[bl][:, :],
                sums[:, bl + 1 : bl + 2].broadcast_to((128, C)),
                ident_s[:, :],
                start=False,
                stop=True,
            ).then_inc(pe_sem, 1)  # pe = BPC + 2

        @block.scalar
        def _(scalar):
            # Per-item PSUM -> SBUF copies on the otherwise idle ACT
            # engine (the last item's copy runs on DVE instead).
            for b in range(BPC - 1):
                scalar.wait_ge(pe_sem, b + 2)
                scalar.activation(
                    out=yt[:, b * F : (b + 1) * F],
                    in_=pts[b][:, :],
                    func=mybir.ActivationFunctionType.Copy,
                ).then_inc(act_sem, 1)

    _NC_CACHE = nc
    return nc


def run_spmd(inputs_arr: np.ndarray, sigma_arr: np.ndarray, trace: bool = False):
    """Shard over batch, run on 8 cores, gather. Returns (out, results_obj)."""
    from concourse import bass_utils

    nc = _build_bass()

    x_full = np.ascontiguousarray(np.asarray(inputs_arr, dtype=np.float32))
    assert x_full.shape == (B, F, T), x_full.shape
    sig = np.asarray(sigma_arr, dtype=np.float32).reshape(1, 1)

    in_maps = [
        {"x": x_full[k * BPC : (k + 1) * BPC], "sig": sig} for k in range(N_CORES)
    ]
    res = bass_utils.run_bass_kernel_spmd(
        nc, in_maps, core_ids=list(range(N_CORES)), trace=trace
    )
    out = np.concatenate([r["y"] for r in res.results], axis=0)
    return out, res


def kernel(**inputs) -> np.ndarray:
    out, _ = run_spmd(inputs["inputs"], inputs["sigma"])
    return out
